# revision 69
# baseline (speedup 1.0000x reference)
"""Trainium2 Bass kernel for nn_DigitCapsules (dynamic-routing capsule layer).

Strategy (per spec sharding_hint): data-parallel over batch B=128 across 8
NeuronCores (16 examples each); dc_w replicated.  Inside each core:

  u[d,bb,n,o] = sum_i x*w runs on the tensor engine via a host-built
  block-diagonal x operand (8 n per matmul group, contraction 64).  Pairs of
  groups are row-packed into the 128x128 array with tile_position (rows 0-63
  and 64-127 compute concurrently), so u-gen streams ~2x faster and DMA uses
  all 128 partitions.

  PSUM is drained into both layouts the routing needs: u1 [p,(d,g,o)]
  (ACT drains; feeds the b-update multiply via the vrep8 broadcast trick
  -> DVE 2x mode) and u2 [p,(d,o,g)] (feeds the c*u multiply with c
  broadcast over o and g innermost -> DVE 2x mode).  Only u2's d0-4 half
  is drained from PSUM on DVE during phase 1; the d5-9 half is an ACT
  transposed copy from u1 overlapped under iteration 1's DVE work (its
  only deadline is mult2-h1, ~34us into the iteration), halving the DVE
  drain load that would otherwise pace phase 1.

  Routing (exact softmax, no per-row max needed):
  ev = exp(b) in f32 (|b| <= ~20 is f32-safe), Z = sum_n ev via a
  per-partition reduce + one broadcast matmul (E2 bb-selector),
  c = ev/Z rounded to f16, s = sum_n c*u.  The whole b -> softmax -> c
  -> s chain is split by d-halves: half 0's weighted fold runs on the
  tensor engine (PSUM-accumulating ones-matmuls, overlapping half 1's
  DVE work); half 1 splits again: d5-7 fold on PE row strips running
  concurrently with a d8-9 DVE g-tree, so neither engine idles in the
  iteration tail.  The b-update fold over o stays on DVE (f16, 2x).

  The s0 fold matmuls are row-split into two concurrent 64-row strips
  (separate psum banks, summed once at iteration 0) so phase 1's PE pace
  drops below the DVE drain pace.
"""

import numpy as np

import concourse.bacc as bacc
import concourse.bass as bass
import concourse.tile as tile
from concourse import mybir
from concourse.bass_utils import run_bass_kernel_spmd

F16 = mybir.dt.float16
F32 = mybir.dt.float32
AF = mybir.ActivationFunctionType

D, B, N, I, O = 10, 128, 1152, 8, 16
NCORES = 8
BB = B // NCORES      # 16
NN = 8                # n's per matmul group
G = N // NN           # 144 groups
GP = G // 2           # 72 row-packed group pairs
DO = D * O            # 160
FU = D * G * O        # 23040 u elements per partition
GCH = 24              # groups per DMA chunk
NCH = G // GCH        # 6
GPC = GCH // 2        # group pairs per chunk
DRAIN = 3             # groups per psum bank (3*160=480 f32)
DBANKS = 2            # banks per drain instruction
DG = D * G            # 1440
SU = G * O            # stride of d in u1/u2 layouts (2304)


def _ap(t, dims, offset=0):
    base = t[:]
    return bass.AP(tensor=base.tensor, offset=base.offset + offset,
                   ap=[base.ap[0]] + [list(d) for d in dims])


def build_nc(debug=False):
    nc = bacc.Bacc(None, target_bir_lowering=False)

    xblk_d = nc.dram_tensor("xblk", [128, GP * NN * BB], F16, kind="ExternalInput")
    wp_d = nc.dram_tensor("wp", [128, GP * DO], F16, kind="ExternalInput")
    eones_d = nc.dram_tensor("eones", [128, 16], F32, kind="ExternalInput")
    e8_d = nc.dram_tensor("e8", [16, 128], F32, kind="ExternalInput")
    e2_d = nc.dram_tensor("e2", [128, 128], F32, kind="ExternalInput")
    out_d = nc.dram_tensor("out", [D, BB, O], F32, kind="ExternalOutput")
    if debug:
        dbg_u1 = nc.dram_tensor("dbg_u1", [128, FU], F16, kind="ExternalOutput")
        dbg_u2 = nc.dram_tensor("dbg_u2", [128, FU], F16, kind="ExternalOutput")
        dbg_t1 = nc.dram_tensor("dbg_t1", [16, DO], F32, kind="ExternalOutput")
        dbg_vv0 = nc.dram_tensor("dbg_vv0", [16, DO], F32, kind="ExternalOutput")
        dbg_b1 = nc.dram_tensor("dbg_b1", [128, DG], F32, kind="ExternalOutput")
        dbg_ev1 = nc.dram_tensor("dbg_ev1", [128, DG], F16, kind="ExternalOutput")
        dbg_sm1 = nc.dram_tensor("dbg_sm1", [16, DO], F32, kind="ExternalOutput")

    with tile.TileContext(nc) as tc:
        with (
            tc.tile_pool(name="const", bufs=1) as const,
            tc.tile_pool(name="big", bufs=1) as big,
            tc.tile_pool(name="stream", bufs=2) as stream,
            tc.tile_pool(name="pmm", bufs=2, space="PSUM") as pmm,
            tc.tile_pool(name="ps0", bufs=1, space="PSUM") as ps0p,
            tc.tile_pool(name="pfold", bufs=1, space="PSUM") as pfoldp,
            tc.tile_pool(name="pvb", bufs=1, space="PSUM") as pvbp,
        ):
            eones = const.tile([128, 16], F32)
            nc.sync.dma_start(eones[:], eones_d[:])
            e8t = const.tile([16, 128], F32)
            nc.sync.dma_start(e8t[:], e8_d[:])
            e2 = const.tile([128, 128], F32)
            nc.sync.dma_start(e2[:], e2_d[:])
            eones16 = const.tile([128, 16], F16)
            nc.scalar.copy(eones16[:], eones[:])

            u1 = big.tile([128, FU], F16)     # (d, g, o)
            u2 = big.tile([128, FU], F16)     # (d, o, g)
            btmp = big.tile([128, FU], F16)   # mult scratch, both layouts
            fbA = big.tile([128, 11520], F16)
            fbB = big.tile([128, 5760], F16)
            vrep8 = big.tile([128, DO * 8], F16)   # (d, g8, o)
            cn16 = big.tile([128, DG], F16)   # normalized softmax weights
            b1 = big.tile([128, DG], F32)
            ub2 = big.tile([128, DG], F32)    # doubles as ev32 = exp(b) f32
            zp = big.tile([128, 16], F32)
            rz128 = big.tile([128, 16], F32)
            ts0 = big.tile([16, 512], F32)
            t0 = big.tile([16, DO], F32)
            t1 = big.tile([16, DO], F32)
            sm = big.tile([16, DO], F32)
            sq = big.tile([16, DO], F32)
            rr = big.tile([16, DO], F32)
            p1 = big.tile([16, DO], F32)
            rden = big.tile([16, DO], F32)
            tt = big.tile([16, DO], F32)
            vv = big.tile([16, DO], F32)

            s0 = ps0p.tile([16, 512], F32, tag="s0")
            s0b = pfoldp.tile([16, 512], F32, tag="pf0")

            def _aps(t, ph, dims, offset=0):
                """AP over a 64-partition slice (row strip ph) of tile t."""
                base = t[ph * 64:(ph + 1) * 64, :]
                return bass.AP(tensor=base.tensor,
                               offset=base.offset + offset,
                               ap=[base.ap[0]] + [list(d) for d in dims])

            # ---------------- phase 1: u generation + s0 fold ----------------
            for ch in range(NCH):
                xch = stream.tile([128, GPC * 128], F16, tag="xch")
                wch = stream.tile([128, GPC * DO], F16, tag="wch")
                nc.sync.dma_start(xch[:], xblk_d[:, ch * GPC * 128:(ch + 1) * GPC * 128])
                nc.sync.dma_start(wch[:], wp_d[:, ch * GPC * DO:(ch + 1) * GPC * DO])
                for dr in range(GCH // (DRAIN * DBANKS)):
                    ps = pmm.tile([128, DBANKS * 512], F32, tag="ps")
                    for gpi in range(3):
                        gpl = dr * 3 + gpi      # group pair within chunk
                        for p in range(2):
                            # bank = parity: the two concurrent row strips
                            # must land in different psum banks.  u carries
                            # a (consistent) permuted g order; all consumers
                            # reduce or broadcast over g, so order is free.
                            bk, j = p, gpi
                            nc.tensor.matmul(
                                _ap(ps, [[DRAIN * O, D], [1, O]],
                                    offset=bk * 512 + j * O),
                                xch[64 * p:64 * p + 64,
                                    gpl * 128:(gpl + 1) * 128],
                                wch[64 * p:64 * p + 64,
                                    gpl * DO:(gpl + 1) * DO],
                                tile_position=(64 * p, 0),
                            )
                    g0 = ch * GCH + dr * DRAIN * DBANKS
                    # u1 drain on ACT (one big copy, both banks)
                    nc.scalar.copy(
                        _ap(u1, [[DRAIN * O, DBANKS], [SU, D], [1, DRAIN * O]],
                            offset=g0 * O),
                        _ap(ps, [[512, DBANKS], [DRAIN * O, D], [1, DRAIN * O]]),
                    )
                    # u2 drains on DVE (transposed to (d, o, g3)) -- only
                    # d0-4: the d5-9 half is not needed until deep into it1,
                    # so it is built later as an ACT copy from u1, halving
                    # the DVE drain load that paces phase 1
                    for bk in range(DBANKS):
                        nc.vector.tensor_copy(
                            _ap(u2, [[SU, 5], [G, O], [1, DRAIN]],
                                offset=g0 + bk * DRAIN),
                            _ap(ps, [[DRAIN * O, 5], [1, O], [O, DRAIN]],
                                offset=bk * 512),
                        )
                # s0 accumulation on PE, delayed one chunk so these fold
                # matmuls (gated on drains) never stall the u-gen stream.
                # Each triple is row-split into two concurrent 64-row strips
                # (separate psum banks) so the fold streams 2 cols/cycle.
                for jt in range(GCH // DRAIN):
                    j = (ch - 1) * (GCH // DRAIN) + jt
                    if j < 0:
                        continue
                    for ph in range(2):
                        nc.tensor.matmul(
                            _ap(s0 if ph == 0 else s0b, [[1, 480]]),
                            eones16[ph * 64:(ph + 1) * 64, :],
                            _aps(u1, ph, [[SU, D], [O, DRAIN], [1, O]],
                                 offset=j * DRAIN * O),
                            start=(j == 0), stop=False,
                            tile_position=(64 * ph, 0),
                            skip_group_check=True,
                        )
            for jt in range(GCH // DRAIN):
                j = (NCH - 1) * (GCH // DRAIN) + jt
                for ph in range(2):
                    nc.tensor.matmul(
                        _ap(s0 if ph == 0 else s0b, [[1, 480]]),
                        eones16[ph * 64:(ph + 1) * 64, :],
                        _aps(u1, ph, [[SU, D], [O, DRAIN], [1, O]],
                             offset=j * DRAIN * O),
                        start=False, stop=(j == G // DRAIN - 1),
                        tile_position=(64 * ph, 0),
                        skip_group_check=True,
                    )

            def squash():
                # vv = sm*|sm|/(1+sm^2)  (== reference squash, safe at sm=0)
                nc.vector.tensor_mul(sq[:], sm[:], sm[:])
                nc.vector.tensor_scalar_mul(tt[:], sm[:], -1.0)
                nc.vector.tensor_max(rr[:], sm[:], tt[:])
                nc.vector.tensor_scalar_add(p1[:], sq[:], 1.0)
                nc.vector.reciprocal(rden[:], p1[:])
                nc.vector.tensor_mul(tt[:], sm[:], rr[:])
                nc.vector.tensor_mul(vv[:], tt[:], rden[:])

            def v_to_vrep8():
                pv = pvbp.tile([128, DO], F32, tag="pvrep")
                nc.tensor.matmul(pv[:], e8t[:], vv[:])
                nc.vector.tensor_copy(
                    _ap(vrep8, [[8 * O, D], [O, 8], [1, O]]),
                    _ap(pv, [[16, D], [0, 8], [1, O]]),
                )

            # ---------------- iteration 0: s0 = mean(u) ----------------
            nc.vector.tensor_copy(ts0[:, 0:480], s0[:, 0:480])
            nc.vector.tensor_add(ts0[:, 0:480], ts0[:, 0:480], s0b[:, 0:480])
            nc.vector.tensor_add(
                _ap(t0, [[O, D], [1, O]]),
                _ap(ts0, [[DRAIN * O, D], [1, O]]),
                _ap(ts0, [[DRAIN * O, D], [1, O]], offset=O),
            )
            nc.vector.tensor_add(
                _ap(t1, [[O, D], [1, O]]),
                _ap(t0, [[O, D], [1, O]]),
                _ap(ts0, [[DRAIN * O, D], [1, O]], offset=2 * O),
            )
            nc.vector.tensor_scalar_mul(sm[:], t1[:], 1.0 / float(N))
            squash()
            v_to_vrep8()
            # u2 d5-9 half: ACT transposed copy from u1, overlapped under
            # it1's DVE mult/fold work (deadline: mult2-h1, ~34us in)
            nc.scalar.copy(
                _ap(u2, [[SU, 3], [G, O], [1, G]], offset=5 * SU),
                _ap(u1, [[SU, 3], [1, O], [O, G]], offset=5 * SU),
            )
            nc.scalar.copy(
                _ap(u2, [[SU, 2], [G, O], [1, G]], offset=8 * SU),
                _ap(u1, [[SU, 2], [1, O], [O, G]], offset=8 * SU),
            )
            if debug:
                nc.sync.dma_start(dbg_u1[:], u1[:])
                nc.sync.dma_start(dbg_t1[:], t1[:])
                nc.sync.dma_start(dbg_vv0[:], vv[:])

            # ---------------- routing iterations 1, 2 ----------------
            for it in (1, 2):
                # mult1: btmp(d,g,o) = u1 * v (broadcast over g via vrep8)
                nc.vector.tensor_mul(
                    _ap(btmp, [[SU, D], [8 * O, G // 8], [1, 8 * O]]),
                    _ap(u1, [[SU, D], [8 * O, G // 8], [1, 8 * O]]),
                    _ap(vrep8, [[8 * O, D], [0, G // 8], [1, 8 * O]]),
                )
                pz = pvbp.tile([128, DO], F32, tag="pvrep")
                pfh = []
                # the whole b -> softmax -> c -> s chain runs per d-half so
                # PE fold matmuls of half 0 overlap DVE work of half 1
                for half in range(2):
                    d0, nd = half * 5, 5
                    # fold over o: 16 -> 8 -> 4 -> 2 -> 1 (last level f32)
                    nc.vector.tensor_add(
                        _ap(fbA, [[G * 8, nd], [8, G], [1, 8]], offset=d0 * G * 8),
                        _ap(btmp, [[SU, nd], [O, G], [1, 8]], offset=d0 * SU),
                        _ap(btmp, [[SU, nd], [O, G], [1, 8]], offset=d0 * SU + 8),
                    )
                    nc.vector.tensor_add(
                        _ap(fbB, [[G * 4, nd], [4, G], [1, 4]], offset=d0 * G * 4),
                        _ap(fbA, [[G * 8, nd], [8, G], [1, 4]], offset=d0 * G * 8),
                        _ap(fbA, [[G * 8, nd], [8, G], [1, 4]],
                            offset=d0 * G * 8 + 4),
                    )
                    nc.vector.tensor_add(
                        _ap(fbA, [[G * 2, nd], [2, G], [1, 2]], offset=d0 * G * 2),
                        _ap(fbB, [[G * 4, nd], [4, G], [1, 2]], offset=d0 * G * 4),
                        _ap(fbB, [[G * 4, nd], [4, G], [1, 2]],
                            offset=d0 * G * 4 + 2),
                    )
                    bdst = b1 if it == 1 else ub2
                    nc.vector.tensor_add(
                        _ap(bdst, [[G, nd], [1, G]], offset=d0 * G),
                        _ap(fbA, [[G * 2, nd], [2, G]], offset=d0 * G * 2),
                        _ap(fbA, [[G * 2, nd], [2, G]], offset=d0 * G * 2 + 1),
                    )
                    if it == 2:
                        nc.vector.tensor_add(
                            _ap(b1, [[1, nd * G]], offset=d0 * G),
                            _ap(b1, [[1, nd * G]], offset=d0 * G),
                            _ap(ub2, [[1, nd * G]], offset=d0 * G),
                        )
                    # exact softmax: ev = exp(b) f32, Z on PE, c = ev/Z f16
                    ev32 = ub2
                    nc.scalar.activation(
                        _ap(ev32, [[1, nd * G]], offset=d0 * G),
                        _ap(b1, [[1, nd * G]], offset=d0 * G), AF.Exp)
                    with nc.allow_low_precision(reason="fp32 accum internally"):
                        nc.vector.reduce_sum(
                            zp[:, d0:d0 + nd],
                            _ap(ev32, [[G, nd], [1, G]], offset=d0 * G),
                            axis=mybir.AxisListType.X,
                        )
                    nc.tensor.matmul(_ap(pz, [[1, nd]], offset=d0),
                                     e2[:], zp[:, d0:d0 + nd])
                    nc.vector.reciprocal(rz128[:, d0:d0 + nd],
                                         _ap(pz, [[1, nd]], offset=d0))
                    nc.vector.tensor_mul(
                        _ap(cn16, [[G, nd], [1, G]], offset=d0 * G),
                        _ap(ev32, [[G, nd], [1, G]], offset=d0 * G),
                        _ap(rz128, [[1, nd], [0, G]], offset=d0),
                    )
                    pf = pfoldp.tile([16, 512], F32, tag=f"pf{half}")
                    pfh.append(pf)
                    nc.vector.tensor_mul(
                        _ap(btmp, [[SU, nd], [G, O], [1, G]], offset=d0 * SU),
                        _ap(u2, [[SU, nd], [G, O], [1, G]], offset=d0 * SU),
                        _ap(cn16, [[G, nd], [0, O], [1, G]], offset=d0 * G),
                    )
                    if half == 0:
                        # PE fold: overlaps the DVE work of half 1
                        for j in range(G // (2 * DRAIN)):
                            nc.tensor.matmul(
                                _ap(pf, [[1, 480]]),
                                eones16[:],
                                _ap(btmp, [[SU, nd], [G, O], [1, 2 * DRAIN]],
                                    offset=d0 * SU + j * 2 * DRAIN),
                                start=(j == 0),
                                stop=(j == G // (2 * DRAIN) - 1),
                                skip_group_check=True,
                            )
                    else:
                        # d5-7 fold on PE row strips (two banks), running
                        # CONCURRENTLY with the d8-9 DVE tree below
                        s0c = ps0p.tile([16, 512], F32, tag="s0")
                        for j in range(G // (2 * DRAIN)):
                            for ph in range(2):
                                nc.tensor.matmul(
                                    _ap(pf if ph == 0 else s0c, [[1, 288]]),
                                    eones16[ph * 64:(ph + 1) * 64, :],
                                    _aps(btmp, ph,
                                         [[SU, 3], [G, O], [1, 2 * DRAIN]],
                                         offset=5 * SU + j * 2 * DRAIN),
                                    start=(j == 0),
                                    stop=(j == G // (2 * DRAIN) - 1),
                                    tile_position=(64 * ph, 0),
                                    skip_group_check=True,
                                )
                        # d8-9 g-fold tree on DVE
                        nc.vector.tensor_add(
                            _ap(fbA, [[1152, 2], [72, O], [1, 72]]),
                            _ap(btmp, [[SU, 2], [G, O], [1, 72]], offset=8 * SU),
                            _ap(btmp, [[SU, 2], [G, O], [1, 72]],
                                offset=8 * SU + 72),
                        )
                        nc.vector.tensor_add(
                            _ap(fbB, [[576, 2], [36, O], [1, 36]]),
                            _ap(fbA, [[1152, 2], [72, O], [1, 36]]),
                            _ap(fbA, [[1152, 2], [72, O], [1, 36]], offset=36),
                        )
                        nc.vector.tensor_add(
                            _ap(fbA, [[288, 2], [18, O], [1, 18]]),
                            _ap(fbB, [[576, 2], [36, O], [1, 18]]),
                            _ap(fbB, [[576, 2], [36, O], [1, 18]], offset=18),
                        )
                        nc.vector.tensor_add(
                            _ap(fbB, [[144, 2], [9, O], [1, 9]]),
                            _ap(fbA, [[288, 2], [18, O], [1, 9]]),
                            _ap(fbA, [[288, 2], [18, O], [1, 9]], offset=9),
                        )
                        with nc.allow_low_precision(reason="f32 accum inside"):
                            nc.vector.reduce_sum(
                                _ap(fbA, [[1, 32]], offset=8000),
                                _ap(fbB, [[144, 2], [9, O], [1, 9]]),
                                axis=mybir.AxisListType.X,
                            )
                        nc.tensor.matmul(
                            _ap(pf, [[1, 32]], offset=288),
                            eones16[:],
                            _ap(fbA, [[1, 32]], offset=8000),
                        )
                # s = sum c*u (c pre-normalized)
                # half 0: psum (d5, o, g6): stage + sum 6 residues
                nc.vector.tensor_copy(ts0[:, 0:480], pfh[0][:, 0:480])
                nc.vector.tensor_add(
                    _ap(ts0, [[6 * O, 5], [6, O], [1, 3]]),
                    _ap(ts0, [[6 * O, 5], [6, O], [1, 3]]),
                    _ap(ts0, [[6 * O, 5], [6, O], [1, 3]], offset=3),
                )
                nc.vector.tensor_add(
                    _ap(t0, [[O, 5], [1, O]]),
                    _ap(ts0, [[6 * O, 5], [6, O]]),
                    _ap(ts0, [[6 * O, 5], [6, O]], offset=1),
                )
                nc.vector.tensor_add(
                    _ap(sm, [[O, 5], [1, O]]),
                    _ap(t0, [[O, 5], [1, O]]),
                    _ap(ts0, [[6 * O, 5], [6, O]], offset=2),
                )
                # half 1 d5-7: stage strip-a, add strip-b, sum 6 residues
                nc.vector.tensor_copy(ts0[:, 0:288], pfh[1][:, 0:288])
                nc.vector.tensor_add(ts0[:, 0:288], ts0[:, 0:288],
                                     s0c[:, 0:288])
                nc.vector.tensor_add(
                    _ap(ts0, [[6 * O, 3], [6, O], [1, 3]]),
                    _ap(ts0, [[6 * O, 3], [6, O], [1, 3]]),
                    _ap(ts0, [[6 * O, 3], [6, O], [1, 3]], offset=3),
                )
                nc.vector.tensor_add(
                    _ap(t0, [[O, 3], [1, O]]),
                    _ap(ts0, [[6 * O, 3], [6, O]]),
                    _ap(ts0, [[6 * O, 3], [6, O]], offset=1),
                )
                nc.vector.tensor_add(
                    _ap(sm, [[O, 3], [1, O]], offset=5 * O),
                    _ap(t0, [[O, 3], [1, O]]),
                    _ap(ts0, [[6 * O, 3], [6, O]], offset=2),
                )
                # half 1 d8-9: tree's ones-matmul result, copied out
                nc.vector.tensor_copy(
                    _ap(sm, [[O, 2], [1, O]], offset=8 * O),
                    _ap(pfh[1], [[O, 2], [1, O]], offset=288),
                )
                squash()
                if debug and it == 1:
                    nc.sync.dma_start(dbg_u2[:], u2[:])
                    nc.sync.dma_start(dbg_b1[:], b1[:])
                    nc.sync.dma_start(dbg_ev1[:], cn16[:])
                    nc.sync.dma_start(dbg_sm1[:], sm[:])
                if it != 2:
                    v_to_vrep8()

            out_ap = bass.AP(tensor=out_d.tensor if hasattr(out_d, "tensor") else out_d,
                             offset=0, ap=[[O, BB], [BB * O, D], [1, O]])
            nc.sync.dma_start(out_ap, vv[:])

    nc.compile()
    return nc


_NC_CACHE = None


def _get_nc():
    global _NC_CACHE
    if _NC_CACHE is None:
        _NC_CACHE = build_nc()
    return _NC_CACHE


def host_prep(x, dc_w):
    x = np.asarray(x, np.float32)
    dc_w = np.asarray(dc_w, np.float32)
    wr = dc_w.reshape(D, G, NN, I, O).transpose(2, 3, 1, 0, 4)   # [nn,i,g,d,o]
    wp64 = np.ascontiguousarray(wr.reshape(64, G, DO)).astype(np.float16)
    # row-pack pairs of g: even g in partitions 0-63, odd in 64-127
    wp = np.concatenate(
        [wp64[:, 0::2, :].reshape(64, GP * DO),
         wp64[:, 1::2, :].reshape(64, GP * DO)], axis=0)
    wp = np.ascontiguousarray(wp)
    xblks = []
    for c in range(NCORES):
        xr = x[c * BB:(c + 1) * BB].reshape(BB, G, NN, I)
        blk = np.zeros((NN, I, G, NN, BB), np.float32)
        for nn in range(NN):
            blk[nn, :, :, nn, :] = xr[:, :, nn, :].transpose(2, 1, 0)
        xb64 = blk.reshape(64, G, NN * BB).astype(np.float16)
        xb = np.concatenate(
            [xb64[:, 0::2, :].reshape(64, GP * NN * BB),
             xb64[:, 1::2, :].reshape(64, GP * NN * BB)], axis=0)
        xblks.append(np.ascontiguousarray(xb))
    eones = np.zeros((128, 16), np.float32)
    for nn in range(NN):
        for bb in range(BB):
            eones[nn * BB + bb, bb] = 1.0
    e8 = np.ascontiguousarray(eones.T)
    e2 = np.ascontiguousarray(eones @ e8)     # [128,128], [bb==bb'] selector
    return wp, xblks, eones, e8, e2


def run(x, dc_w, **spmd_kwargs):
    wp, xblks, eones, e8, e2 = host_prep(x, dc_w)
    nc = _get_nc()
    in_maps = [
        {"xblk": xblks[c], "wp": wp, "eones": eones, "e8": e8, "e2": e2}
        for c in range(NCORES)
    ]
    res = run_bass_kernel_spmd(nc, in_maps, core_ids=list(range(NCORES)), **spmd_kwargs)
    out = np.zeros((D, B, 1, 1, O), np.float32)
    for c in range(NCORES):
        out[:, c * BB:(c + 1) * BB, 0, 0, :] = res.results[c]["out"]
    return out, res


def kernel(x, dc_w):
    return run(x, dc_w)[0]


# revision 71
# speedup vs baseline: 1.2027x; 1.2027x over previous
"""Trainium2 Bass kernel for nn_DigitCapsules (dynamic-routing capsule layer).

Strategy (per spec sharding_hint): data-parallel over batch B=128 across 8
NeuronCores (16 examples each); dc_w replicated.  Inside each core:

  u[d,bb,n,o] = sum_i x*w runs on the tensor engine via a host-built
  block-diagonal x operand (8 n per matmul group, contraction 64).  Pairs of
  groups are row-packed into the 128x128 array with tile_position (rows 0-63
  and 64-127 compute concurrently), so u-gen streams ~2x faster and DMA uses
  all 128 partitions.

  PSUM is drained into both layouts the routing needs: u1 [p,(d,g,o)]
  (ACT drains; feeds the b-update multiply via the vrep8 broadcast trick
  -> DVE 2x mode) and u2 [p,(d,o,g)] (feeds the c*u multiply with c
  broadcast over o and g innermost -> DVE 2x mode).  Only u2's d0-4 half
  is drained from PSUM on DVE during phase 1; the d5-9 half is an ACT
  transposed copy from u1 overlapped under iteration 1's DVE work (its
  only deadline is mult2-h1, ~34us into the iteration), halving the DVE
  drain load that would otherwise pace phase 1.

  Routing (exact softmax, no per-row max needed):
  ev = exp(b) in f32 (|b| <= ~20 is f32-safe), Z = sum_n ev via a
  per-partition reduce + one broadcast matmul (E2 bb-selector),
  c = ev/Z rounded to f16, s = sum_n c*u.  The whole b -> softmax -> c
  -> s chain is split by d-halves: half 0's weighted fold runs on the
  tensor engine (PSUM-accumulating ones-matmuls, overlapping half 1's
  DVE work); half 1 splits again: d5-7 fold on PE row strips running
  concurrently with a d8-9 DVE g-tree, so neither engine idles in the
  iteration tail.  The b-update fold over o stays on DVE (f16, 2x).

  The s0 fold matmuls are row-split into two concurrent 64-row strips
  (separate psum banks, summed once at iteration 0) so phase 1's PE pace
  drops below the DVE drain pace.
"""

import numpy as np

import concourse.bacc as bacc
import concourse.bass as bass
import concourse.tile as tile
from concourse import mybir
from concourse.bass_utils import run_bass_kernel_spmd

F16 = mybir.dt.float16
F32 = mybir.dt.float32
AF = mybir.ActivationFunctionType

D, B, N, I, O = 10, 128, 1152, 8, 16
NCORES = 8
BB = B // NCORES      # 16
NN = 8                # n's per matmul group
G = N // NN           # 144 groups
GP = G // 2           # 72 row-packed group pairs
DO = D * O            # 160
FU = D * G * O        # 23040 u elements per partition
GCH = 12              # groups per DMA chunk
NCH = G // GCH        # 12
DRAIN = 3             # groups per psum bank (3*160=480 f32)
DBANKS = 2            # banks per drain instruction
DG = D * G            # 1440
SU = G * O            # stride of d in u1/u2 layouts (2304)


def _ap(t, dims, offset=0):
    base = t[:]
    return bass.AP(tensor=base.tensor, offset=base.offset + offset,
                   ap=[base.ap[0]] + [list(d) for d in dims])


def build_nc(debug=False):
    nc = bacc.Bacc(None, target_bir_lowering=False)

    xblk_d = nc.dram_tensor("xblk", [128, GP * NN * BB], F16, kind="ExternalInput")
    wp_d = nc.dram_tensor("wp", [128, GP * DO], F16, kind="ExternalInput")
    eones_d = nc.dram_tensor("eones", [128, 16], F32, kind="ExternalInput")
    e8_d = nc.dram_tensor("e8", [16, 128], F32, kind="ExternalInput")
    e2_d = nc.dram_tensor("e2", [128, 128], F32, kind="ExternalInput")
    out_d = nc.dram_tensor("out", [D, BB, O], F32, kind="ExternalOutput")
    if debug:
        dbg_u1 = nc.dram_tensor("dbg_u1", [128, FU], F16, kind="ExternalOutput")
        dbg_u2 = nc.dram_tensor("dbg_u2", [128, FU], F16, kind="ExternalOutput")
        dbg_t1 = nc.dram_tensor("dbg_t1", [16, DO], F32, kind="ExternalOutput")
        dbg_vv0 = nc.dram_tensor("dbg_vv0", [16, DO], F32, kind="ExternalOutput")
        dbg_b1 = nc.dram_tensor("dbg_b1", [128, DG], F32, kind="ExternalOutput")
        dbg_ev1 = nc.dram_tensor("dbg_ev1", [128, DG], F16, kind="ExternalOutput")
        dbg_sm1 = nc.dram_tensor("dbg_sm1", [16, DO], F32, kind="ExternalOutput")

    with tile.TileContext(nc) as tc:
        with (
            tc.tile_pool(name="const", bufs=1) as const,
            tc.tile_pool(name="big", bufs=1) as big,
            tc.tile_pool(name="stream", bufs=3) as stream,
            tc.tile_pool(name="pmm", bufs=2, space="PSUM") as pmm,
            tc.tile_pool(name="ps0", bufs=1, space="PSUM") as ps0p,
            tc.tile_pool(name="pfold", bufs=1, space="PSUM") as pfoldp,
            tc.tile_pool(name="pvb", bufs=1, space="PSUM") as pvbp,
        ):
            eones = const.tile([128, 16], F32)
            nc.sync.dma_start(eones[:], eones_d[:])
            e8t = const.tile([16, 128], F32)
            nc.sync.dma_start(e8t[:], e8_d[:])
            e2 = const.tile([128, 128], F32)
            nc.sync.dma_start(e2[:], e2_d[:])
            eones16 = const.tile([128, 16], F16)
            nc.scalar.copy(eones16[:], eones[:])

            u1 = big.tile([128, FU], F16)     # (d, g, o)
            u2 = big.tile([128, FU], F16)     # (d, o, g)
            btmp = big.tile([128, FU], F16)   # mult scratch, both layouts
            fbA = big.tile([128, 11520], F16)
            fbB = big.tile([128, 5760], F16)
            v16 = big.tile([128, DO], F16)    # v broadcast to (nn,bb)
            cn16 = big.tile([128, DG], F16)   # normalized softmax weights
            b1 = big.tile([128, DG], F32)
            ub2 = big.tile([128, DG], F32)    # doubles as ev32 = exp(b) f32
            zp = big.tile([128, 16], F32)
            rz128 = big.tile([128, 16], F32)
            ts0 = big.tile([16, 512], F32)
            t0 = big.tile([16, DO], F32)
            t1 = big.tile([16, DO], F32)
            sm = big.tile([16, DO], F32)
            sq = big.tile([16, DO], F32)
            rr = big.tile([16, DO], F32)
            p1 = big.tile([16, DO], F32)
            rden = big.tile([16, DO], F32)
            tt = big.tile([16, DO], F32)
            vv = big.tile([16, DO], F32)

            s0 = ps0p.tile([16, 512], F32, tag="s0")
            s0b = pfoldp.tile([16, 512], F32, tag="pf0")

            def _aps(t, ph, dims, offset=0):
                """AP over a 64-partition slice (row strip ph) of tile t."""
                base = t[ph * 64:(ph + 1) * 64, :]
                return bass.AP(tensor=base.tensor,
                               offset=base.offset + offset,
                               ap=[base.ap[0]] + [list(d) for d in dims])

            # ---------------- phase 1: u generation + s0 fold ----------------
            for ch in range(NCH):
                xch = stream.tile([128, 6 * 128], F16, tag="xch")
                wch = stream.tile([128, 6 * DO], F16, tag="wch")
                nc.sync.dma_start(xch[:], xblk_d[:, ch * 6 * 128:(ch + 1) * 6 * 128])
                nc.sync.dma_start(wch[:], wp_d[:, ch * 6 * DO:(ch + 1) * 6 * DO])
                for dr in range(GCH // (DRAIN * DBANKS)):
                    ps = pmm.tile([128, DBANKS * 512], F32, tag="ps")
                    for gpi in range(3):
                        gpl = dr * 3 + gpi      # group pair within chunk
                        for p in range(2):
                            # bank = parity: the two concurrent row strips
                            # must land in different psum banks.  u carries
                            # a (consistent) permuted g order; all consumers
                            # reduce or broadcast over g, so order is free.
                            bk, j = p, gpi
                            nc.tensor.matmul(
                                _ap(ps, [[DRAIN * O, D], [1, O]],
                                    offset=bk * 512 + j * O),
                                xch[64 * p:64 * p + 64,
                                    gpl * 128:(gpl + 1) * 128],
                                wch[64 * p:64 * p + 64,
                                    gpl * DO:(gpl + 1) * DO],
                                tile_position=(64 * p, 0),
                            )
                    g0 = ch * GCH + dr * DRAIN * DBANKS
                    # u1 drain on ACT (one big copy, both banks)
                    nc.scalar.copy(
                        _ap(u1, [[DRAIN * O, DBANKS], [SU, D], [1, DRAIN * O]],
                            offset=g0 * O),
                        _ap(ps, [[512, DBANKS], [DRAIN * O, D], [1, DRAIN * O]]),
                    )
                    # u2 drains on DVE (transposed to (d, o, g3)) -- only
                    # d0-4: the d5-9 half is not needed until deep into it1,
                    # so it is built later as an ACT copy from u1, halving
                    # the DVE drain load that paces phase 1
                    for bk in range(DBANKS):
                        nc.vector.tensor_copy(
                            _ap(u2, [[SU, 5], [G, O], [1, DRAIN]],
                                offset=g0 + bk * DRAIN),
                            _ap(ps, [[DRAIN * O, 5], [1, O], [O, DRAIN]],
                                offset=bk * 512),
                        )
                # s0 accumulation on PE, delayed one chunk so these fold
                # matmuls (gated on drains) never stall the u-gen stream.
                # Each triple is row-split into two concurrent 64-row strips
                # (separate psum banks) so the fold streams 2 cols/cycle.
                for jt in range(GCH // DRAIN):
                    j = (ch - 1) * (GCH // DRAIN) + jt
                    if j < 0:
                        continue
                    for ph in range(2):
                        nc.tensor.matmul(
                            _ap(s0 if ph == 0 else s0b, [[1, 480]]),
                            eones16[ph * 64:(ph + 1) * 64, :],
                            _aps(u1, ph, [[SU, D], [O, DRAIN], [1, O]],
                                 offset=j * DRAIN * O),
                            start=(j == 0), stop=False,
                            tile_position=(64 * ph, 0),
                            skip_group_check=True,
                        )
            for jt in range(GCH // DRAIN):
                j = (NCH - 1) * (GCH // DRAIN) + jt
                for ph in range(2):
                    nc.tensor.matmul(
                        _ap(s0 if ph == 0 else s0b, [[1, 480]]),
                        eones16[ph * 64:(ph + 1) * 64, :],
                        _aps(u1, ph, [[SU, D], [O, DRAIN], [1, O]],
                             offset=j * DRAIN * O),
                        start=False, stop=(j == G // DRAIN - 1),
                        tile_position=(64 * ph, 0),
                        skip_group_check=True,
                    )

            def squash():
                # vv = sm*|sm|/(1+sm^2)  (== reference squash, safe at sm=0)
                nc.vector.tensor_mul(sq[:], sm[:], sm[:])
                nc.vector.tensor_scalar_mul(tt[:], sm[:], -1.0)
                nc.vector.tensor_max(rr[:], sm[:], tt[:])
                nc.vector.tensor_scalar_add(p1[:], sq[:], 1.0)
                nc.vector.reciprocal(rden[:], p1[:])
                nc.vector.tensor_mul(tt[:], sm[:], rr[:])
                nc.vector.tensor_mul(vv[:], tt[:], rden[:])

            def v_to_vrep8():
                pv = pvbp.tile([128, DO], F32, tag="pvrep")
                nc.tensor.matmul(pv[:], e8t[:], vv[:])
                nc.vector.tensor_copy(v16[:], pv[:])

            # ---------------- iteration 0: s0 = mean(u) ----------------
            nc.vector.tensor_copy(ts0[:, 0:480], s0[:, 0:480])
            nc.vector.tensor_add(ts0[:, 0:480], ts0[:, 0:480], s0b[:, 0:480])
            nc.vector.tensor_add(
                _ap(t0, [[O, D], [1, O]]),
                _ap(ts0, [[DRAIN * O, D], [1, O]]),
                _ap(ts0, [[DRAIN * O, D], [1, O]], offset=O),
            )
            nc.vector.tensor_add(
                _ap(t1, [[O, D], [1, O]]),
                _ap(t0, [[O, D], [1, O]]),
                _ap(ts0, [[DRAIN * O, D], [1, O]], offset=2 * O),
            )
            nc.vector.tensor_scalar_mul(sm[:], t1[:], 1.0 / float(N))
            squash()
            v_to_vrep8()
            # u2 d5-9 half: ACT transposed copy from u1, overlapped under
            # it1's DVE mult/fold work (deadline: mult2-h1, ~34us in)
            nc.scalar.copy(
                _ap(u2, [[SU, 3], [G, O], [1, G]], offset=5 * SU),
                _ap(u1, [[SU, 3], [1, O], [O, G]], offset=5 * SU),
            )
            nc.scalar.copy(
                _ap(u2, [[SU, 2], [G, O], [1, G]], offset=8 * SU),
                _ap(u1, [[SU, 2], [1, O], [O, G]], offset=8 * SU),
            )
            if debug:
                nc.sync.dma_start(dbg_u1[:], u1[:])
                nc.sync.dma_start(dbg_t1[:], t1[:])
                nc.sync.dma_start(dbg_vv0[:], vv[:])

            # ---------------- routing iterations 1, 2 ----------------
            for it in (1, 2):
                # mult1: btmp(d,g,o) = u1 * v (broadcast over g via vrep8)
                nc.vector.tensor_mul(
                    _ap(btmp, [[SU, D], [O, G], [1, O]]),
                    _ap(u1, [[SU, D], [O, G], [1, O]]),
                    _ap(v16, [[O, D], [0, G], [1, O]]),
                )
                pz = pvbp.tile([128, DO], F32, tag="pvrep")
                pfh = []
                # the whole b -> softmax -> c -> s chain runs per d-half so
                # PE fold matmuls of half 0 overlap DVE work of half 1
                for half in range(2):
                    d0, nd = half * 5, 5
                    # fold over o: 16 -> 8 -> 4 -> 2 -> 1 (last level f32)
                    nc.vector.tensor_add(
                        _ap(fbA, [[G * 8, nd], [8, G], [1, 8]], offset=d0 * G * 8),
                        _ap(btmp, [[SU, nd], [O, G], [1, 8]], offset=d0 * SU),
                        _ap(btmp, [[SU, nd], [O, G], [1, 8]], offset=d0 * SU + 8),
                    )
                    nc.vector.tensor_add(
                        _ap(fbB, [[G * 4, nd], [4, G], [1, 4]], offset=d0 * G * 4),
                        _ap(fbA, [[G * 8, nd], [8, G], [1, 4]], offset=d0 * G * 8),
                        _ap(fbA, [[G * 8, nd], [8, G], [1, 4]],
                            offset=d0 * G * 8 + 4),
                    )
                    nc.vector.tensor_add(
                        _ap(fbA, [[G * 2, nd], [2, G], [1, 2]], offset=d0 * G * 2),
                        _ap(fbB, [[G * 4, nd], [4, G], [1, 2]], offset=d0 * G * 4),
                        _ap(fbB, [[G * 4, nd], [4, G], [1, 2]],
                            offset=d0 * G * 4 + 2),
                    )
                    bdst = b1 if it == 1 else ub2
                    nc.vector.tensor_add(
                        _ap(bdst, [[G, nd], [1, G]], offset=d0 * G),
                        _ap(fbA, [[G * 2, nd], [2, G]], offset=d0 * G * 2),
                        _ap(fbA, [[G * 2, nd], [2, G]], offset=d0 * G * 2 + 1),
                    )
                    if it == 2:
                        nc.vector.tensor_add(
                            _ap(b1, [[1, nd * G]], offset=d0 * G),
                            _ap(b1, [[1, nd * G]], offset=d0 * G),
                            _ap(ub2, [[1, nd * G]], offset=d0 * G),
                        )
                    # exact softmax: ev = exp(b) f32, Z on PE, c = ev/Z f16
                    ev32 = ub2
                    nc.scalar.activation(
                        _ap(ev32, [[1, nd * G]], offset=d0 * G),
                        _ap(b1, [[1, nd * G]], offset=d0 * G), AF.Exp)
                    with nc.allow_low_precision(reason="fp32 accum internally"):
                        nc.vector.reduce_sum(
                            zp[:, d0:d0 + nd],
                            _ap(ev32, [[G, nd], [1, G]], offset=d0 * G),
                            axis=mybir.AxisListType.X,
                        )
                    nc.tensor.matmul(_ap(pz, [[1, nd]], offset=d0),
                                     e2[:], zp[:, d0:d0 + nd])
                    nc.vector.reciprocal(rz128[:, d0:d0 + nd],
                                         _ap(pz, [[1, nd]], offset=d0))
                    nc.vector.tensor_mul(
                        _ap(cn16, [[G, nd], [1, G]], offset=d0 * G),
                        _ap(ev32, [[G, nd], [1, G]], offset=d0 * G),
                        _ap(rz128, [[1, nd], [0, G]], offset=d0),
                    )
                    pf = pfoldp.tile([16, 512], F32, tag=f"pf{half}")
                    pfh.append(pf)
                    nc.vector.tensor_mul(
                        _ap(btmp, [[SU, nd], [G, O], [1, G]], offset=d0 * SU),
                        _ap(u2, [[SU, nd], [G, O], [1, G]], offset=d0 * SU),
                        _ap(cn16, [[G, nd], [0, O], [1, G]], offset=d0 * G),
                    )
                    if half == 0:
                        # PE fold: overlaps the DVE work of half 1
                        for j in range(G // (2 * DRAIN)):
                            nc.tensor.matmul(
                                _ap(pf, [[1, 480]]),
                                eones16[:],
                                _ap(btmp, [[SU, nd], [G, O], [1, 2 * DRAIN]],
                                    offset=d0 * SU + j * 2 * DRAIN),
                                start=(j == 0),
                                stop=(j == G // (2 * DRAIN) - 1),
                                skip_group_check=True,
                            )
                    else:
                        # d5-7 fold on PE row strips (two banks), running
                        # CONCURRENTLY with the d8-9 DVE tree below
                        s0c = ps0p.tile([16, 512], F32, tag="s0")
                        for j in range(G // (2 * DRAIN)):
                            for ph in range(2):
                                nc.tensor.matmul(
                                    _ap(pf if ph == 0 else s0c, [[1, 288]]),
                                    eones16[ph * 64:(ph + 1) * 64, :],
                                    _aps(btmp, ph,
                                         [[SU, 3], [G, O], [1, 2 * DRAIN]],
                                         offset=5 * SU + j * 2 * DRAIN),
                                    start=(j == 0),
                                    stop=(j == G // (2 * DRAIN) - 1),
                                    tile_position=(64 * ph, 0),
                                    skip_group_check=True,
                                )
                        # d8-9 g-fold tree on DVE
                        nc.vector.tensor_add(
                            _ap(fbA, [[1152, 2], [72, O], [1, 72]]),
                            _ap(btmp, [[SU, 2], [G, O], [1, 72]], offset=8 * SU),
                            _ap(btmp, [[SU, 2], [G, O], [1, 72]],
                                offset=8 * SU + 72),
                        )
                        nc.vector.tensor_add(
                            _ap(fbB, [[576, 2], [36, O], [1, 36]]),
                            _ap(fbA, [[1152, 2], [72, O], [1, 36]]),
                            _ap(fbA, [[1152, 2], [72, O], [1, 36]], offset=36),
                        )
                        nc.vector.tensor_add(
                            _ap(fbA, [[288, 2], [18, O], [1, 18]]),
                            _ap(fbB, [[576, 2], [36, O], [1, 18]]),
                            _ap(fbB, [[576, 2], [36, O], [1, 18]], offset=18),
                        )
                        nc.vector.tensor_add(
                            _ap(fbB, [[144, 2], [9, O], [1, 9]]),
                            _ap(fbA, [[288, 2], [18, O], [1, 9]]),
                            _ap(fbA, [[288, 2], [18, O], [1, 9]], offset=9),
                        )
                        with nc.allow_low_precision(reason="f32 accum inside"):
                            nc.vector.reduce_sum(
                                _ap(fbA, [[1, 32]], offset=8000),
                                _ap(fbB, [[144, 2], [9, O], [1, 9]]),
                                axis=mybir.AxisListType.X,
                            )
                        nc.tensor.matmul(
                            _ap(pf, [[1, 32]], offset=288),
                            eones16[:],
                            _ap(fbA, [[1, 32]], offset=8000),
                        )
                # s = sum c*u (c pre-normalized)
                # half 0: psum (d5, o, g6): stage + sum 6 residues
                nc.vector.tensor_copy(ts0[:, 0:480], pfh[0][:, 0:480])
                nc.vector.tensor_add(
                    _ap(ts0, [[6 * O, 5], [6, O], [1, 3]]),
                    _ap(ts0, [[6 * O, 5], [6, O], [1, 3]]),
                    _ap(ts0, [[6 * O, 5], [6, O], [1, 3]], offset=3),
                )
                nc.vector.tensor_add(
                    _ap(t0, [[O, 5], [1, O]]),
                    _ap(ts0, [[6 * O, 5], [6, O]]),
                    _ap(ts0, [[6 * O, 5], [6, O]], offset=1),
                )
                nc.vector.tensor_add(
                    _ap(sm, [[O, 5], [1, O]]),
                    _ap(t0, [[O, 5], [1, O]]),
                    _ap(ts0, [[6 * O, 5], [6, O]], offset=2),
                )
                # half 1 d5-7: stage strip-a, add strip-b, sum 6 residues
                nc.vector.tensor_copy(ts0[:, 0:288], pfh[1][:, 0:288])
                nc.vector.tensor_add(ts0[:, 0:288], ts0[:, 0:288],
                                     s0c[:, 0:288])
                nc.vector.tensor_add(
                    _ap(ts0, [[6 * O, 3], [6, O], [1, 3]]),
                    _ap(ts0, [[6 * O, 3], [6, O], [1, 3]]),
                    _ap(ts0, [[6 * O, 3], [6, O], [1, 3]], offset=3),
                )
                nc.vector.tensor_add(
                    _ap(t0, [[O, 3], [1, O]]),
                    _ap(ts0, [[6 * O, 3], [6, O]]),
                    _ap(ts0, [[6 * O, 3], [6, O]], offset=1),
                )
                nc.vector.tensor_add(
                    _ap(sm, [[O, 3], [1, O]], offset=5 * O),
                    _ap(t0, [[O, 3], [1, O]]),
                    _ap(ts0, [[6 * O, 3], [6, O]], offset=2),
                )
                # half 1 d8-9: tree's ones-matmul result, copied out
                nc.vector.tensor_copy(
                    _ap(sm, [[O, 2], [1, O]], offset=8 * O),
                    _ap(pfh[1], [[O, 2], [1, O]], offset=288),
                )
                squash()
                if debug and it == 1:
                    nc.sync.dma_start(dbg_u2[:], u2[:])
                    nc.sync.dma_start(dbg_b1[:], b1[:])
                    nc.sync.dma_start(dbg_ev1[:], cn16[:])
                    nc.sync.dma_start(dbg_sm1[:], sm[:])
                if it != 2:
                    v_to_vrep8()

            out_ap = bass.AP(tensor=out_d.tensor if hasattr(out_d, "tensor") else out_d,
                             offset=0, ap=[[O, BB], [BB * O, D], [1, O]])
            nc.sync.dma_start(out_ap, vv[:])

    nc.compile()
    return nc


_NC_CACHE = None


def _get_nc():
    global _NC_CACHE
    if _NC_CACHE is None:
        _NC_CACHE = build_nc()
    return _NC_CACHE


def host_prep(x, dc_w):
    x = np.asarray(x, np.float32)
    dc_w = np.asarray(dc_w, np.float32)
    wr = dc_w.reshape(D, G, NN, I, O).transpose(2, 3, 1, 0, 4)   # [nn,i,g,d,o]
    wp64 = np.ascontiguousarray(wr.reshape(64, G, DO)).astype(np.float16)
    # row-pack pairs of g: even g in partitions 0-63, odd in 64-127
    wp = np.concatenate(
        [wp64[:, 0::2, :].reshape(64, GP * DO),
         wp64[:, 1::2, :].reshape(64, GP * DO)], axis=0)
    wp = np.ascontiguousarray(wp)
    xblks = []
    for c in range(NCORES):
        xr = x[c * BB:(c + 1) * BB].reshape(BB, G, NN, I)
        blk = np.zeros((NN, I, G, NN, BB), np.float32)
        for nn in range(NN):
            blk[nn, :, :, nn, :] = xr[:, :, nn, :].transpose(2, 1, 0)
        xb64 = blk.reshape(64, G, NN * BB).astype(np.float16)
        xb = np.concatenate(
            [xb64[:, 0::2, :].reshape(64, GP * NN * BB),
             xb64[:, 1::2, :].reshape(64, GP * NN * BB)], axis=0)
        xblks.append(np.ascontiguousarray(xb))
    eones = np.zeros((128, 16), np.float32)
    for nn in range(NN):
        for bb in range(BB):
            eones[nn * BB + bb, bb] = 1.0
    e8 = np.ascontiguousarray(eones.T)
    e2 = np.ascontiguousarray(eones @ e8)     # [128,128], [bb==bb'] selector
    return wp, xblks, eones, e8, e2


def run(x, dc_w, **spmd_kwargs):
    wp, xblks, eones, e8, e2 = host_prep(x, dc_w)
    nc = _get_nc()
    in_maps = [
        {"xblk": xblks[c], "wp": wp, "eones": eones, "e8": e8, "e2": e2}
        for c in range(NCORES)
    ]
    res = run_bass_kernel_spmd(nc, in_maps, core_ids=list(range(NCORES)), **spmd_kwargs)
    out = np.zeros((D, B, 1, 1, O), np.float32)
    for c in range(NCORES):
        out[:, c * BB:(c + 1) * BB, 0, 0, :] = res.results[c]["out"]
    return out, res


def kernel(x, dc_w):
    return run(x, dc_w)[0]
